# revision 7
# baseline (speedup 1.0000x reference)
"""Trainium2 Bass kernel for nn_ButterflyLinear.

Computes y = x @ (mask * W)^T + bias with
  x: (8, 2048, 1024) f32, W/mask: (4096, 1024) f32, bias: (4096,) f32.

Strategy:
  - Data-parallel over the batch dim: core c computes batch element c
    (2048 tokens x 4096 out-features).
  - The butterfly mask is banded: at (128 out x 128 in) block granularity
    only ~70/256 blocks are nonzero.  The block-occupancy pattern is
    derived on the host from the actual mask input, and only occupied
    blocks of W / mask are loaded and multiplied on device.
  - Matmul layout: out[tok=128, of=512] = lhsT[if=128, tok=128]^T @
    rhs[if=128, of=512], accumulated over the occupied if-chunks of each
    of-group.  Both x tiles and masked-W tiles are transposed on-chip via
    the PE transpose (contraction dim must sit on SBUF partitions).
  - Matmuls run in float32r mode (full fp32 data; fast PE mode at N>=512).
  - bias is folded into the PSUM->SBUF eviction as a DVE tensor_add
    against a host-broadcast bias tile.
"""

import numpy as np

import concourse.bass as bass
import concourse.bacc as bacc
import concourse.mybir as mybir
from concourse.tile import TileContext
from concourse.masks import make_identity
from concourse.bass_utils import run_bass_kernel_spmd

N_CORES = 8
B, S, IN_F, OUT_F = 8, 2048, 1024, 4096
P = 128                      # partition block
NG_COLS = 512                # matmul moving free dim (of-group width)
N_IB = IN_F // P             # 8 input-feature chunks
N_OB = OUT_F // P            # 32 out-feature blocks
N_G = OUT_F // NG_COLS       # 8 of-groups
OB_PER_G = NG_COLS // P      # 4 out-blocks per of-group

# Matmul dtype: float32r = fp32 data in the PE's fast "replicated" mode
# (1 cycle/row at N>=256 vs 4 cycles/row for plain float32).
MM_DTYPE = mybir.dt.float32r

_program_cache: dict = {}


def _block_occupancy(sparse_mask: np.ndarray) -> np.ndarray:
    """(N_OB, N_IB) bool: which (128 out x 128 in) blocks have any nonzero."""
    blocks = sparse_mask.reshape(N_OB, P, N_IB, P)
    return (blocks != 0).any(axis=(1, 3))


def _build_program(occ_key: bytes, tok: int):
    """Build + compile the per-core Bass program.

    occ_key: packed bool (N_OB, N_IB) block occupancy.
    tok: tokens per core (2048 for the real problem).
    """
    occ = np.frombuffer(occ_key, dtype=bool).reshape(N_OB, N_IB)
    n_tt = tok // P  # token tiles

    # Per out-block: contiguous span of occupied if-chunks (band).
    ob_band = {}
    for ob in range(N_OB):
        ibs = np.where(occ[ob])[0]
        if len(ibs):
            ob_band[ob] = (int(ibs.min()), int(ibs.max()))
    # Per of-group: occupied if-chunks (union over the group's 4 out-blocks).
    g_ibs = {}
    for g in range(N_G):
        s = sorted({ib for ob in range(g * OB_PER_G, (g + 1) * OB_PER_G)
                    for ib in np.where(occ[ob])[0].tolist()})
        g_ibs[g] = s
    # Column slot in the WmT store for each (g, ib) pair.
    pair_slot = {}
    for g in range(N_G):
        for ib in g_ibs[g]:
            pair_slot[(g, ib)] = len(pair_slot)
    n_pairs = len(pair_slot)

    nc = bacc.Bacc("TRN2", target_bir_lowering=False, debug=False,
                   num_devices=N_CORES)
    f32 = mybir.dt.float32
    x_d = nc.dram_tensor("x", [tok, IN_F], f32, kind="ExternalInput").ap()
    w_d = nc.dram_tensor("w", [OUT_F, IN_F], f32, kind="ExternalInput").ap()
    m_d = nc.dram_tensor("m", [OUT_F, IN_F], f32, kind="ExternalInput").ap()
    b_d = nc.dram_tensor("b", [P, OUT_F], f32, kind="ExternalInput").ap()
    y_d = nc.dram_tensor("y", [tok, OUT_F], f32, kind="ExternalOutput").ap()

    with TileContext(nc) as tc:
        with (
            tc.tile_pool(name="const", bufs=1) as const_pool,
            tc.tile_pool(name="wprep", bufs=3) as wprep_pool,
            tc.tile_pool(name="xio", bufs=3) as xio_pool,
            tc.tile_pool(name="yio", bufs=2) as yio_pool,
            tc.tile_pool(name="psum_tr", bufs=2, space="PSUM") as ptr_pool,
            tc.tile_pool(name="psum_mm", bufs=3, space="PSUM") as pmm_pool,
        ):
            identity = const_pool.tile([P, P], f32)
            make_identity(nc, identity[:])

            bias_bc = const_pool.tile([P, OUT_F], f32)
            nc.sync.dma_start(out=bias_bc[:], in_=b_d[:, :])

            zblk = const_pool.tile([P, P], f32)
            nc.vector.memset(zblk[:], 0.0)

            # Persistent store of transposed masked weights:
            # column slot k holds WmT[if=128, of=512] for pair k.
            # Stored as MM_DTYPE: the PSUM->SBUF copy rounds to fp32r,
            # which the fp32r matmul requires of its producers.
            wmT = const_pool.tile([P, n_pairs * NG_COLS], MM_DTYPE)

            # ---- W prep: load occupied bands, mask, transpose ----
            for ob in range(N_OB):
                if ob not in ob_band:
                    continue
                ib0, ib1 = ob_band[ob]
                nb = ib1 - ib0 + 1
                g, sub = ob // OB_PER_G, ob % OB_PER_G
                wblk = wprep_pool.tile([P, nb * P], f32, tag="wblk")
                mblk = wprep_pool.tile([P, nb * P], f32, tag="mblk")
                r0, r1 = ob * P, (ob + 1) * P
                c0, c1 = ib0 * P, (ib1 + 1) * P
                nc.sync.dma_start(out=wblk[:], in_=w_d[r0:r1, c0:c1])
                nc.sync.dma_start(out=mblk[:], in_=m_d[r0:r1, c0:c1])
                nc.vector.tensor_mul(wblk[:], wblk[:], mblk[:])
                for ib in range(ib0, ib1 + 1):
                    col = pair_slot[(g, ib)] * NG_COLS + sub * P
                    if not occ[ob, ib]:
                        # this 128-col slot of the pair is all-zero
                        nc.scalar.copy(out=wmT[:, col:col + P], in_=zblk[:])
                        continue
                    pst = ptr_pool.tile([P, P], f32, tag="ptr")
                    nc.tensor.transpose(
                        pst[:], wblk[:, (ib - ib0) * P:(ib - ib0 + 1) * P],
                        identity[:])
                    nc.scalar.copy(out=wmT[:, col:col + P], in_=pst[:])
            # zero slots for (g, ib) pairs where an out-block has no band row
            for g in range(N_G):
                for ib in g_ibs[g]:
                    for sub in range(OB_PER_G):
                        ob = g * OB_PER_G + sub
                        in_band = ob in ob_band and \
                            ob_band[ob][0] <= ib <= ob_band[ob][1]
                        if not in_band:
                            col = pair_slot[(g, ib)] * NG_COLS + sub * P
                            nc.scalar.copy(out=wmT[:, col:col + P],
                                           in_=zblk[:])

            # ---- main loop over token tiles ----
            for t in range(n_tt):
                xtile = xio_pool.tile([P, IN_F], f32, tag="xtile")
                nc.sync.dma_start(out=xtile[:],
                                  in_=x_d[t * P:(t + 1) * P, :])
                xT = xio_pool.tile([P, IN_F], MM_DTYPE, tag="xT")
                for ib in range(N_IB):
                    pst = ptr_pool.tile([P, P], f32, tag="ptr")
                    nc.tensor.transpose(
                        pst[:], xtile[:, ib * P:(ib + 1) * P], identity[:])
                    nc.scalar.copy(out=xT[:, ib * P:(ib + 1) * P], in_=pst[:])

                ytile = yio_pool.tile([P, OUT_F], f32, tag="ytile")
                for g in range(N_G):
                    ibs = g_ibs[g]
                    pmm = pmm_pool.tile([P, NG_COLS], f32, tag="pmm")
                    for j, ib in enumerate(ibs):
                        sl = pair_slot[(g, ib)] * NG_COLS
                        nc.tensor.matmul(
                            pmm[:],
                            xT[:, ib * P:(ib + 1) * P],
                            wmT[:, sl:sl + NG_COLS],
                            start=(j == 0), stop=(j == len(ibs) - 1))
                    nc.vector.tensor_add(
                        ytile[:, g * NG_COLS:(g + 1) * NG_COLS],
                        pmm[:],
                        bias_bc[:, g * NG_COLS:(g + 1) * NG_COLS])
                nc.sync.dma_start(out=y_d[t * P:(t + 1) * P, :], in_=ytile[:])

    nc.compile()
    return nc


def get_program(sparse_mask: np.ndarray, tok: int = S):
    occ = _block_occupancy(sparse_mask)
    key = (occ.tobytes(), tok)
    if key not in _program_cache:
        _program_cache[key] = _build_program(occ.tobytes(), tok)
    return _program_cache[key]


def make_in_maps(x, weight, bias, sparse_mask):
    bias_bc = np.ascontiguousarray(
        np.broadcast_to(bias.reshape(1, OUT_F), (P, OUT_F))).astype(np.float32)
    w = np.ascontiguousarray(weight, dtype=np.float32)
    m = np.ascontiguousarray(sparse_mask, dtype=np.float32)
    return [
        {
            "x": np.ascontiguousarray(x[c], dtype=np.float32),
            "w": w,
            "m": m,
            "b": bias_bc,
        }
        for c in range(N_CORES)
    ]


def kernel(x, weight, bias, sparse_mask):
    x = np.asarray(x)
    weight = np.asarray(weight)
    bias = np.asarray(bias)
    sparse_mask = np.asarray(sparse_mask)
    assert x.shape == (B, S, IN_F), x.shape
    assert weight.shape == (OUT_F, IN_F)
    assert sparse_mask.shape == (OUT_F, IN_F)

    nc = get_program(sparse_mask)
    in_maps = make_in_maps(x, weight, bias, sparse_mask)
    res = run_bass_kernel_spmd(nc, in_maps, core_ids=list(range(N_CORES)))
    y = np.stack([res.results[c]["y"] for c in range(N_CORES)], axis=0)
    return y.astype(np.float32)


# revision 13
# speedup vs baseline: 6.8036x; 6.8036x over previous
"""Trainium2 Bass kernel for nn_ButterflyLinear.

Computes y = x @ (mask * W)^T + bias with
  x: (8, 2048, 1024) f32, W/mask: (4096, 1024) f32, bias: (4096,) f32.

Strategy:
  - Data-parallel over the batch dim: core c computes batch element c
    (2048 tokens x 4096 out-features).
  - The butterfly mask is banded: at (128 out x 128 in) block granularity
    only ~70/256 blocks are nonzero.  The block-occupancy pattern is
    derived on the host from the actual mask input, and only occupied
    blocks of W / mask are loaded and multiplied on device.
  - Matmul layout: out[tok=128, of=512] = lhsT[if=128, tok=128]^T @
    rhs[if=128, of=512], accumulated over the occupied if-chunks of each
    of-group.  Both x tiles and masked-W tiles are transposed on-chip via
    the PE transpose (contraction dim must sit on SBUF partitions).
  - Matmuls run in plain float32 (measured faster than float32r on this
    hardware, and bit-accurate).  y-stores issue from the GpSimd (SWDGE)
    queue so store waits don't head-of-line-block input loads on SP.
  - bias is folded into the PSUM->SBUF eviction as a DVE tensor_add
    against a host-broadcast bias tile.
"""

import numpy as np

import concourse.bass as bass
import concourse.bacc as bacc
import concourse.mybir as mybir
from concourse.tile import TileContext
from concourse.masks import make_identity
from concourse.bass_utils import run_bass_kernel_spmd

N_CORES = 8
B, S, IN_F, OUT_F = 8, 2048, 1024, 4096
P = 128                      # partition block
NG_COLS = 512                # matmul moving free dim (of-group width)
N_IB = IN_F // P             # 8 input-feature chunks
N_OB = OUT_F // P            # 32 out-feature blocks
N_G = OUT_F // NG_COLS       # 8 of-groups
OB_PER_G = NG_COLS // P      # 4 out-blocks per of-group

# Matmul dtype. Plain float32 measured faster than float32r on this
# hardware (and is bit-accurate: rel err ~8e-8 vs ~1.4e-4 for fp32r).
MM_DTYPE = mybir.dt.float32

# Production variant: fp32 matmuls; y-stores issued from the (otherwise
# idle) GpSimd queue so store waits never head-of-line-block the input
# loads issued from the SP queue.
DEFAULT_VARIANT = "f32mm,ygp"

_program_cache: dict = {}


def _block_occupancy(sparse_mask: np.ndarray) -> np.ndarray:
    """(N_OB, N_IB) bool: which (128 out x 128 in) blocks have any nonzero."""
    blocks = sparse_mask.reshape(N_OB, P, N_IB, P)
    return (blocks != 0).any(axis=(1, 3))


def _build_program(occ_key: bytes, tok: int, variant: str = ""):
    """Build + compile the per-core Bass program.

    occ_key: packed bool (N_OB, N_IB) block occupancy.
    tok: tokens per core (2048 for the real problem).
    variant: ablation flags, comma-separated subset of
      {nomm, noydma, noxdma, f32mm, nowprep}.
    """
    flags = set(variant.split(",")) if variant else set()
    mm_dtype = mybir.dt.float32 if "f32mm" in flags else MM_DTYPE
    occ = np.frombuffer(occ_key, dtype=bool).reshape(N_OB, N_IB)
    n_tt = tok // P  # token tiles

    # Per out-block: contiguous span of occupied if-chunks (band).
    ob_band = {}
    for ob in range(N_OB):
        ibs = np.where(occ[ob])[0]
        if len(ibs):
            ob_band[ob] = (int(ibs.min()), int(ibs.max()))
    # Per of-group: occupied if-chunks (union over the group's 4 out-blocks).
    g_ibs = {}
    for g in range(N_G):
        s = sorted({ib for ob in range(g * OB_PER_G, (g + 1) * OB_PER_G)
                    for ib in np.where(occ[ob])[0].tolist()})
        g_ibs[g] = s
    # Column slot in the WmT store for each (g, ib) pair.
    pair_slot = {}
    for g in range(N_G):
        for ib in g_ibs[g]:
            pair_slot[(g, ib)] = len(pair_slot)
    n_pairs = len(pair_slot)

    nc = bacc.Bacc("TRN2", target_bir_lowering=False, debug=False,
                   num_devices=N_CORES)
    f32 = mybir.dt.float32
    x_d = nc.dram_tensor("x", [tok, IN_F], f32, kind="ExternalInput").ap()
    if "hostwm" in flags:
        w_d = nc.dram_tensor("wm", [OUT_F, IN_F], f32,
                             kind="ExternalInput").ap()
        m_d = None
    else:
        w_d = nc.dram_tensor("w", [OUT_F, IN_F], f32,
                             kind="ExternalInput").ap()
        m_d = nc.dram_tensor("m", [OUT_F, IN_F], f32,
                             kind="ExternalInput").ap()
    b_d = nc.dram_tensor("b", [P, OUT_F], f32, kind="ExternalInput").ap()
    y_d = nc.dram_tensor("y", [tok, OUT_F], f32, kind="ExternalOutput").ap()

    with TileContext(nc) as tc:
        with (
            tc.tile_pool(name="const", bufs=1) as const_pool,
            tc.tile_pool(name="wprep", bufs=3) as wprep_pool,
            tc.tile_pool(name="xio", bufs=3) as xio_pool,
            tc.tile_pool(name="yio", bufs=2) as yio_pool,
            tc.tile_pool(name="psum_tr", bufs=2, space="PSUM") as ptr_pool,
            tc.tile_pool(name="psum_mm", bufs=3, space="PSUM") as pmm_pool,
        ):
            identity = const_pool.tile([P, P], f32)
            make_identity(nc, identity[:])

            bias_bc = const_pool.tile([P, OUT_F], f32)
            nc.sync.dma_start(out=bias_bc[:], in_=b_d[:, :])

            zblk = const_pool.tile([P, P], f32)
            nc.vector.memset(zblk[:], 0.0)

            # Persistent store of transposed masked weights:
            # column slot k holds WmT[if=128, of=512] for pair k.
            # Stored as MM_DTYPE: the PSUM->SBUF copy rounds to fp32r,
            # which the fp32r matmul requires of its producers.
            wmT = const_pool.tile([P, n_pairs * NG_COLS], mm_dtype)

            # ---- W prep: load occupied bands, mask, transpose ----
            if "gload" in flags and "nowprep" not in flags:
                # one coalesced DMA per of-group: 4 out-blocks stacked
                # along the free dim, covering the group's band union
                for g in range(N_G):
                    ibs = g_ibs[g]
                    if not ibs:
                        continue
                    gib0, gib1 = min(ibs), max(ibs)
                    nbg = gib1 - gib0 + 1
                    wblk = wprep_pool.tile([P, OB_PER_G * nbg * P], f32,
                                           tag="wblk")
                    src_w = w_d[g * NG_COLS:(g + 1) * NG_COLS,
                                gib0 * P:(gib1 + 1) * P].rearrange(
                                    "(a p) c -> p (a c)", p=P)
                    nc.sync.dma_start(out=wblk[:], in_=src_w)
                    if m_d is not None:
                        mblk = wprep_pool.tile([P, OB_PER_G * nbg * P], f32,
                                               tag="mblk")
                        src_m = m_d[g * NG_COLS:(g + 1) * NG_COLS,
                                    gib0 * P:(gib1 + 1) * P].rearrange(
                                        "(a p) c -> p (a c)", p=P)
                        nc.sync.dma_start(out=mblk[:], in_=src_m)
                        nc.vector.tensor_mul(wblk[:], wblk[:], mblk[:])
                    for sub in range(OB_PER_G):
                        ob = g * OB_PER_G + sub
                        for ib in ibs:
                            col = pair_slot[(g, ib)] * NG_COLS + sub * P
                            if not occ[ob, ib]:
                                nc.scalar.copy(out=wmT[:, col:col + P],
                                               in_=zblk[:])
                                continue
                            off = (sub * nbg + (ib - gib0)) * P
                            pst = ptr_pool.tile([P, P], f32, tag="ptr")
                            nc.tensor.transpose(
                                pst[:], wblk[:, off:off + P], identity[:])
                            nc.scalar.copy(out=wmT[:, col:col + P],
                                           in_=pst[:])
            else:
                for ob in ([] if "nowprep" in flags else range(N_OB)):
                    if ob not in ob_band:
                        continue
                    ib0, ib1 = ob_band[ob]
                    nb = ib1 - ib0 + 1
                    g, sub = ob // OB_PER_G, ob % OB_PER_G
                    wblk = wprep_pool.tile([P, nb * P], f32, tag="wblk")
                    r0, r1 = ob * P, (ob + 1) * P
                    c0, c1 = ib0 * P, (ib1 + 1) * P
                    nc.sync.dma_start(out=wblk[:], in_=w_d[r0:r1, c0:c1])
                    if m_d is not None:
                        mblk = wprep_pool.tile([P, nb * P], f32, tag="mblk")
                        nc.sync.dma_start(out=mblk[:], in_=m_d[r0:r1, c0:c1])
                        nc.vector.tensor_mul(wblk[:], wblk[:], mblk[:])
                    for ib in range(ib0, ib1 + 1):
                        col = pair_slot[(g, ib)] * NG_COLS + sub * P
                        if not occ[ob, ib]:
                            # this 128-col slot of the pair is all-zero
                            nc.scalar.copy(out=wmT[:, col:col + P],
                                           in_=zblk[:])
                            continue
                        pst = ptr_pool.tile([P, P], f32, tag="ptr")
                        nc.tensor.transpose(
                            pst[:], wblk[:, (ib - ib0) * P:(ib - ib0 + 1) * P],
                            identity[:])
                        nc.scalar.copy(out=wmT[:, col:col + P], in_=pst[:])
            # zero slots for (g, ib) pairs where an out-block has no band row
            # (gload covers every (sub, ib in g_ibs) slot already)
            for g in ([] if ("nowprep" in flags or "gload" in flags)
                      else range(N_G)):
                for ib in g_ibs[g]:
                    for sub in range(OB_PER_G):
                        ob = g * OB_PER_G + sub
                        in_band = ob in ob_band and \
                            ob_band[ob][0] <= ib <= ob_band[ob][1]
                        if not in_band:
                            col = pair_slot[(g, ib)] * NG_COLS + sub * P
                            nc.scalar.copy(out=wmT[:, col:col + P],
                                           in_=zblk[:])

            # ---- main loop over token tiles ----
            for t in range(n_tt):
                xtile = xio_pool.tile([P, IN_F], f32, tag="xtile")
                if "noxdma" not in flags:
                    nc.sync.dma_start(out=xtile[:],
                                      in_=x_d[t * P:(t + 1) * P, :])
                else:
                    nc.vector.memset(xtile[:], 0.5)
                xT = xio_pool.tile([P, IN_F], mm_dtype, tag="xT")
                if "xt512" in flags:
                    for q in range(N_IB // 4):
                        pst4 = ptr_pool.tile([P, 4 * P], f32, tag="ptr4")
                        for k in range(4):
                            ib = q * 4 + k
                            nc.tensor.transpose(
                                pst4[:, k * P:(k + 1) * P],
                                xtile[:, ib * P:(ib + 1) * P], identity[:])
                        nc.scalar.copy(out=xT[:, q * 4 * P:(q + 1) * 4 * P],
                                       in_=pst4[:])
                else:
                    for ib in range(N_IB):
                        pst = ptr_pool.tile([P, P], f32, tag="ptr")
                        nc.tensor.transpose(
                            pst[:], xtile[:, ib * P:(ib + 1) * P], identity[:])
                        nc.scalar.copy(out=xT[:, ib * P:(ib + 1) * P],
                                       in_=pst[:])

                ytile = yio_pool.tile([P, OUT_F], f32, tag="ytile")
                for g in range(N_G):
                    ibs = g_ibs[g]
                    if "nomm" in flags:
                        nc.vector.tensor_copy(
                            ytile[:, g * NG_COLS:(g + 1) * NG_COLS],
                            bias_bc[:, g * NG_COLS:(g + 1) * NG_COLS])
                        continue
                    pmm = pmm_pool.tile([P, NG_COLS], f32, tag="pmm")
                    for j, ib in enumerate(ibs):
                        sl = pair_slot[(g, ib)] * NG_COLS
                        nc.tensor.matmul(
                            pmm[:],
                            xT[:, ib * P:(ib + 1) * P],
                            wmT[:, sl:sl + NG_COLS],
                            start=(j == 0), stop=(j == len(ibs) - 1))
                    nc.vector.tensor_add(
                        ytile[:, g * NG_COLS:(g + 1) * NG_COLS],
                        pmm[:],
                        bias_bc[:, g * NG_COLS:(g + 1) * NG_COLS])
                if "noydma" not in flags:
                    ydma = nc.gpsimd if "ygp" in flags else nc.sync
                    ydma.dma_start(out=y_d[t * P:(t + 1) * P, :],
                                   in_=ytile[:])

    nc.compile()
    return nc


def get_program(sparse_mask: np.ndarray, tok: int = S, variant: str = ""):
    occ = _block_occupancy(sparse_mask)
    key = (occ.tobytes(), tok, variant)
    if key not in _program_cache:
        _program_cache[key] = _build_program(occ.tobytes(), tok, variant)
    return _program_cache[key]


def make_in_maps(x, weight, bias, sparse_mask, variant: str = ""):
    flags = set(variant.split(",")) if variant else set()
    bias_bc = np.ascontiguousarray(
        np.broadcast_to(bias.reshape(1, OUT_F), (P, OUT_F))).astype(np.float32)
    if "hostwm" in flags:
        wm = np.ascontiguousarray(
            np.asarray(weight, np.float32) * np.asarray(sparse_mask,
                                                        np.float32))
        base = {"wm": wm, "b": bias_bc}
    else:
        base = {
            "w": np.ascontiguousarray(weight, dtype=np.float32),
            "m": np.ascontiguousarray(sparse_mask, dtype=np.float32),
            "b": bias_bc,
        }
    return [
        {"x": np.ascontiguousarray(x[c], dtype=np.float32), **base}
        for c in range(N_CORES)
    ]


def kernel(x, weight, bias, sparse_mask):
    x = np.asarray(x)
    weight = np.asarray(weight)
    bias = np.asarray(bias)
    sparse_mask = np.asarray(sparse_mask)
    assert x.shape == (B, S, IN_F), x.shape
    assert weight.shape == (OUT_F, IN_F)
    assert sparse_mask.shape == (OUT_F, IN_F)

    nc = get_program(sparse_mask, variant=DEFAULT_VARIANT)
    in_maps = make_in_maps(x, weight, bias, sparse_mask,
                           variant=DEFAULT_VARIANT)
    res = run_bass_kernel_spmd(nc, in_maps, core_ids=list(range(N_CORES)))
    y = np.stack([res.results[c]["y"] for c in range(N_CORES)], axis=0)
    return y.astype(np.float32)


# revision 14
# speedup vs baseline: 8.1398x; 1.1964x over previous
"""Trainium2 Bass kernel for nn_ButterflyLinear.

Computes y = x @ (mask * W)^T + bias with
  x: (8, 2048, 1024) f32, W/mask: (4096, 1024) f32, bias: (4096,) f32.

Strategy:
  - Data-parallel over the batch dim: core c computes batch element c
    (2048 tokens x 4096 out-features).
  - The butterfly mask is banded: at (128 out x 128 in) block granularity
    only ~70/256 blocks are nonzero.  The block-occupancy pattern is
    derived on the host from the actual mask input, and only occupied
    blocks of W / mask are loaded and multiplied on device.
  - Matmul layout: out[tok=128, of=512] = lhsT[if=128, tok=128]^T @
    rhs[if=128, of=512], accumulated over the occupied if-chunks of each
    of-group.  Both x tiles and masked-W tiles are transposed on-chip via
    the PE transpose (contraction dim must sit on SBUF partitions).
  - Matmuls run in plain float32 (measured faster than float32r on this
    hardware, and bit-accurate).  y-stores issue from the GpSimd (SWDGE)
    queue so store waits don't head-of-line-block input loads on SP.
  - bias is folded into the PSUM->SBUF eviction as a DVE tensor_add
    against a host-broadcast bias tile.
"""

import numpy as np

import concourse.bass as bass
import concourse.bacc as bacc
import concourse.mybir as mybir
from concourse.tile import TileContext
from concourse.masks import make_identity
from concourse.bass_utils import run_bass_kernel_spmd

N_CORES = 8
B, S, IN_F, OUT_F = 8, 2048, 1024, 4096
P = 128                      # partition block
NG_COLS = 512                # matmul moving free dim (of-group width)
N_IB = IN_F // P             # 8 input-feature chunks
N_OB = OUT_F // P            # 32 out-feature blocks
N_G = OUT_F // NG_COLS       # 8 of-groups
OB_PER_G = NG_COLS // P      # 4 out-blocks per of-group

# Matmul dtype. Plain float32 measured faster than float32r on this
# hardware (and is bit-accurate: rel err ~8e-8 vs ~1.4e-4 for fp32r).
MM_DTYPE = mybir.dt.float32

# Production variant: fp32 matmuls; y-stores issued from the (otherwise
# idle) GpSimd queue so store waits never head-of-line-block the input
# loads issued from the SP queue.
DEFAULT_VARIANT = "f32mm,ygp"

_program_cache: dict = {}


def _block_occupancy(sparse_mask: np.ndarray) -> np.ndarray:
    """(N_OB, N_IB) bool: which (128 out x 128 in) blocks have any nonzero."""
    blocks = sparse_mask.reshape(N_OB, P, N_IB, P)
    return (blocks != 0).any(axis=(1, 3))


def _build_program(occ_key: bytes, tok: int, variant: str = ""):
    """Build + compile the per-core Bass program.

    occ_key: packed bool (N_OB, N_IB) block occupancy.
    tok: tokens per core (2048 for the real problem).
    variant: ablation flags, comma-separated subset of
      {nomm, noydma, noxdma, f32mm, nowprep}.
    """
    flags = set(variant.split(",")) if variant else set()
    mm_dtype = mybir.dt.float32 if "f32mm" in flags else MM_DTYPE
    occ = np.frombuffer(occ_key, dtype=bool).reshape(N_OB, N_IB)
    n_tt = tok // P  # token tiles

    # Per out-block: contiguous span of occupied if-chunks (band).
    ob_band = {}
    for ob in range(N_OB):
        ibs = np.where(occ[ob])[0]
        if len(ibs):
            ob_band[ob] = (int(ibs.min()), int(ibs.max()))
    # Per of-group: occupied if-chunks (union over the group's 4 out-blocks).
    g_ibs = {}
    for g in range(N_G):
        s = sorted({ib for ob in range(g * OB_PER_G, (g + 1) * OB_PER_G)
                    for ib in np.where(occ[ob])[0].tolist()})
        g_ibs[g] = s
    # Column slot in the WmT store for each (g, ib) pair.
    pair_slot = {}
    for g in range(N_G):
        for ib in g_ibs[g]:
            pair_slot[(g, ib)] = len(pair_slot)
    n_pairs = len(pair_slot)

    nc = bacc.Bacc("TRN2", target_bir_lowering=False, debug=False,
                   num_devices=N_CORES)
    f32 = mybir.dt.float32
    x_d = nc.dram_tensor("x", [tok, IN_F], f32, kind="ExternalInput").ap()
    if "hostwm" in flags:
        w_d = nc.dram_tensor("wm", [OUT_F, IN_F], f32,
                             kind="ExternalInput").ap()
        m_d = None
    else:
        w_d = nc.dram_tensor("w", [OUT_F, IN_F], f32,
                             kind="ExternalInput").ap()
        m_d = nc.dram_tensor("m", [OUT_F, IN_F], f32,
                             kind="ExternalInput").ap()
    b_d = nc.dram_tensor("b", [P, OUT_F], f32, kind="ExternalInput").ap()
    y_d = nc.dram_tensor("y", [tok, OUT_F], f32, kind="ExternalOutput").ap()

    with TileContext(nc) as tc:
        with (
            tc.tile_pool(name="const", bufs=1) as const_pool,
            tc.tile_pool(name="wprep", bufs=3) as wprep_pool,
            tc.tile_pool(name="xio", bufs=3) as xio_pool,
            tc.tile_pool(name="yio", bufs=2) as yio_pool,
            tc.tile_pool(name="psum_tr", bufs=2, space="PSUM") as ptr_pool,
            tc.tile_pool(name="psum_mm", bufs=3, space="PSUM") as pmm_pool,
        ):
            identity = const_pool.tile([P, P], f32)
            make_identity(nc, identity[:])

            bias_bc = const_pool.tile([P, OUT_F], f32)
            nc.sync.dma_start(out=bias_bc[:], in_=b_d[:, :])

            zblk = const_pool.tile([P, P], f32)
            nc.vector.memset(zblk[:], 0.0)

            # Persistent store of transposed masked weights:
            # column slot k holds WmT[if=128, of=512] for pair k.
            # Stored as MM_DTYPE: the PSUM->SBUF copy rounds to fp32r,
            # which the fp32r matmul requires of its producers.
            wmT = const_pool.tile([P, n_pairs * NG_COLS], mm_dtype)

            # ---- W prep: load occupied bands, mask, transpose ----
            if "gload" in flags and "nowprep" not in flags:
                # one coalesced DMA per of-group: 4 out-blocks stacked
                # along the free dim, covering the group's band union
                for g in range(N_G):
                    ibs = g_ibs[g]
                    if not ibs:
                        continue
                    gib0, gib1 = min(ibs), max(ibs)
                    nbg = gib1 - gib0 + 1
                    wblk = wprep_pool.tile([P, OB_PER_G * nbg * P], f32,
                                           tag="wblk")
                    src_w = w_d[g * NG_COLS:(g + 1) * NG_COLS,
                                gib0 * P:(gib1 + 1) * P].rearrange(
                                    "(a p) c -> p (a c)", p=P)
                    nc.sync.dma_start(out=wblk[:], in_=src_w)
                    if m_d is not None:
                        mblk = wprep_pool.tile([P, OB_PER_G * nbg * P], f32,
                                               tag="mblk")
                        src_m = m_d[g * NG_COLS:(g + 1) * NG_COLS,
                                    gib0 * P:(gib1 + 1) * P].rearrange(
                                        "(a p) c -> p (a c)", p=P)
                        nc.sync.dma_start(out=mblk[:], in_=src_m)
                        nc.vector.tensor_mul(wblk[:], wblk[:], mblk[:])
                    for sub in range(OB_PER_G):
                        ob = g * OB_PER_G + sub
                        for ib in ibs:
                            col = pair_slot[(g, ib)] * NG_COLS + sub * P
                            if not occ[ob, ib]:
                                nc.scalar.copy(out=wmT[:, col:col + P],
                                               in_=zblk[:])
                                continue
                            off = (sub * nbg + (ib - gib0)) * P
                            pst = ptr_pool.tile([P, P], f32, tag="ptr")
                            nc.tensor.transpose(
                                pst[:], wblk[:, off:off + P], identity[:])
                            nc.scalar.copy(out=wmT[:, col:col + P],
                                           in_=pst[:])
            else:
                for ob in ([] if "nowprep" in flags else range(N_OB)):
                    if ob not in ob_band:
                        continue
                    ib0, ib1 = ob_band[ob]
                    nb = ib1 - ib0 + 1
                    g, sub = ob // OB_PER_G, ob % OB_PER_G
                    wblk = wprep_pool.tile([P, nb * P], f32, tag="wblk")
                    r0, r1 = ob * P, (ob + 1) * P
                    c0, c1 = ib0 * P, (ib1 + 1) * P
                    nc.sync.dma_start(out=wblk[:], in_=w_d[r0:r1, c0:c1])
                    if m_d is not None:
                        mblk = wprep_pool.tile([P, nb * P], f32, tag="mblk")
                        nc.sync.dma_start(out=mblk[:], in_=m_d[r0:r1, c0:c1])
                        nc.vector.tensor_mul(wblk[:], wblk[:], mblk[:])
                    for ib in range(ib0, ib1 + 1):
                        col = pair_slot[(g, ib)] * NG_COLS + sub * P
                        if not occ[ob, ib]:
                            # this 128-col slot of the pair is all-zero
                            nc.scalar.copy(out=wmT[:, col:col + P],
                                           in_=zblk[:])
                            continue
                        pst = ptr_pool.tile([P, P], f32, tag="ptr")
                        nc.tensor.transpose(
                            pst[:], wblk[:, (ib - ib0) * P:(ib - ib0 + 1) * P],
                            identity[:])
                        nc.scalar.copy(out=wmT[:, col:col + P], in_=pst[:])
            # zero slots for (g, ib) pairs where an out-block has no band row
            # (gload covers every (sub, ib in g_ibs) slot already)
            for g in ([] if ("nowprep" in flags or "gload" in flags)
                      else range(N_G)):
                for ib in g_ibs[g]:
                    for sub in range(OB_PER_G):
                        ob = g * OB_PER_G + sub
                        in_band = ob in ob_band and \
                            ob_band[ob][0] <= ib <= ob_band[ob][1]
                        if not in_band:
                            col = pair_slot[(g, ib)] * NG_COLS + sub * P
                            nc.scalar.copy(out=wmT[:, col:col + P],
                                           in_=zblk[:])

            # ---- main loop over token tiles ----
            for t in range(n_tt):
                xtile = xio_pool.tile([P, IN_F], f32, tag="xtile")
                if "noxdma" not in flags:
                    nc.sync.dma_start(out=xtile[:],
                                      in_=x_d[t * P:(t + 1) * P, :])
                else:
                    nc.vector.memset(xtile[:], 0.5)
                xT = xio_pool.tile([P, IN_F], mm_dtype, tag="xT")
                if "xt512" in flags:
                    for q in range(N_IB // 4):
                        pst4 = ptr_pool.tile([P, 4 * P], f32, tag="ptr4")
                        for k in range(4):
                            ib = q * 4 + k
                            nc.tensor.transpose(
                                pst4[:, k * P:(k + 1) * P],
                                xtile[:, ib * P:(ib + 1) * P], identity[:])
                        nc.scalar.copy(out=xT[:, q * 4 * P:(q + 1) * 4 * P],
                                       in_=pst4[:])
                else:
                    for ib in range(N_IB):
                        pst = ptr_pool.tile([P, P], f32, tag="ptr")
                        nc.tensor.transpose(
                            pst[:], xtile[:, ib * P:(ib + 1) * P], identity[:])
                        nc.scalar.copy(out=xT[:, ib * P:(ib + 1) * P],
                                       in_=pst[:])

                ytile = yio_pool.tile([P, OUT_F], f32, tag="ytile")
                for g in range(N_G):
                    ibs = g_ibs[g]
                    if not ibs:
                        # of-group with no occupied blocks: output is bias
                        nc.vector.tensor_copy(
                            ytile[:, g * NG_COLS:(g + 1) * NG_COLS],
                            bias_bc[:, g * NG_COLS:(g + 1) * NG_COLS])
                        continue
                    if "nomm" in flags:
                        nc.vector.tensor_copy(
                            ytile[:, g * NG_COLS:(g + 1) * NG_COLS],
                            bias_bc[:, g * NG_COLS:(g + 1) * NG_COLS])
                        continue
                    pmm = pmm_pool.tile([P, NG_COLS], f32, tag="pmm")
                    for j, ib in enumerate(ibs):
                        sl = pair_slot[(g, ib)] * NG_COLS
                        nc.tensor.matmul(
                            pmm[:],
                            xT[:, ib * P:(ib + 1) * P],
                            wmT[:, sl:sl + NG_COLS],
                            start=(j == 0), stop=(j == len(ibs) - 1))
                    nc.vector.tensor_add(
                        ytile[:, g * NG_COLS:(g + 1) * NG_COLS],
                        pmm[:],
                        bias_bc[:, g * NG_COLS:(g + 1) * NG_COLS])
                if "noydma" not in flags:
                    ydma = nc.gpsimd if "ygp" in flags else nc.sync
                    ydma.dma_start(out=y_d[t * P:(t + 1) * P, :],
                                   in_=ytile[:])

    nc.compile()
    return nc


def get_program(sparse_mask: np.ndarray, tok: int = S, variant: str = ""):
    occ = _block_occupancy(sparse_mask)
    key = (occ.tobytes(), tok, variant)
    if key not in _program_cache:
        _program_cache[key] = _build_program(occ.tobytes(), tok, variant)
    return _program_cache[key]


def make_in_maps(x, weight, bias, sparse_mask, variant: str = ""):
    flags = set(variant.split(",")) if variant else set()
    bias_bc = np.ascontiguousarray(
        np.broadcast_to(bias.reshape(1, OUT_F), (P, OUT_F))).astype(np.float32)
    if "hostwm" in flags:
        wm = np.ascontiguousarray(
            np.asarray(weight, np.float32) * np.asarray(sparse_mask,
                                                        np.float32))
        base = {"wm": wm, "b": bias_bc}
    else:
        base = {
            "w": np.ascontiguousarray(weight, dtype=np.float32),
            "m": np.ascontiguousarray(sparse_mask, dtype=np.float32),
            "b": bias_bc,
        }
    return [
        {"x": np.ascontiguousarray(x[c], dtype=np.float32), **base}
        for c in range(N_CORES)
    ]


def kernel(x, weight, bias, sparse_mask):
    x = np.asarray(x)
    weight = np.asarray(weight)
    bias = np.asarray(bias)
    sparse_mask = np.asarray(sparse_mask)
    assert x.shape == (B, S, IN_F), x.shape
    assert weight.shape == (OUT_F, IN_F)
    assert sparse_mask.shape == (OUT_F, IN_F)

    nc = get_program(sparse_mask, variant=DEFAULT_VARIANT)
    in_maps = make_in_maps(x, weight, bias, sparse_mask,
                           variant=DEFAULT_VARIANT)
    res = run_bass_kernel_spmd(nc, in_maps, core_ids=list(range(N_CORES)))
    y = np.stack([res.results[c]["y"] for c in range(N_CORES)], axis=0)
    return y.astype(np.float32)


# revision 17
# speedup vs baseline: 48.9953x; 6.0192x over previous
"""Trainium2 Bass kernel for nn_ButterflyLinear.

Computes y = x @ (mask * W)^T + bias with
  x: (8, 2048, 1024) f32, W/mask: (4096, 1024) f32, bias: (4096,) f32.

Strategy:
  - Data-parallel over the batch dim: core c computes batch element c
    (2048 tokens x 4096 out-features).
  - The butterfly mask is banded: at (128 out x 128 in) block granularity
    only ~70/256 blocks are nonzero.  The block-occupancy pattern is
    derived on the host from the actual mask input, and only occupied
    blocks of W / mask are loaded and multiplied on device.
  - Matmul layout: out[tok=128, of=512] = lhsT[if=128, tok=128]^T @
    rhs[if=128, of=512], accumulated over the occupied if-chunks of each
    of-group.  Both x tiles and masked-W tiles are transposed on-chip via
    the PE transpose (contraction dim must sit on SBUF partitions).
  - Matmuls run in plain float32 (measured faster than float32r on this
    hardware, and bit-accurate).  y-stores issue from the GpSimd (SWDGE)
    queue so store waits don't head-of-line-block input loads on SP.
  - bias is folded into the PSUM->SBUF eviction as a DVE tensor_add
    against a host-broadcast bias tile.
"""

import numpy as np

import concourse.bass as bass
import concourse.bacc as bacc
import concourse.mybir as mybir
from concourse.tile import TileContext
from concourse.masks import make_identity
from concourse.bass_utils import run_bass_kernel_spmd

N_CORES = 8
B, S, IN_F, OUT_F = 8, 2048, 1024, 4096
P = 128                      # partition block
NG_COLS = 512                # matmul moving free dim (of-group width)
N_IB = IN_F // P             # 8 input-feature chunks
N_OB = OUT_F // P            # 32 out-feature blocks
N_G = OUT_F // NG_COLS       # 8 of-groups
OB_PER_G = NG_COLS // P      # 4 out-blocks per of-group

# Matmul dtype. Plain float32 measured faster than float32r on this
# hardware (and is bit-accurate: rel err ~8e-8 vs ~1.4e-4 for fp32r).
MM_DTYPE = mybir.dt.float32

# Production variant: fp32 matmuls; y-stores issued from the (otherwise
# idle) GpSimd queue so store waits never head-of-line-block the input
# loads issued from the SP queue.
DEFAULT_VARIANT = "f32mm,ygp"

_program_cache: dict = {}


def _block_occupancy(sparse_mask: np.ndarray) -> np.ndarray:
    """(N_OB, N_IB) bool: which (128 out x 128 in) blocks have any nonzero."""
    blocks = sparse_mask.reshape(N_OB, P, N_IB, P)
    return (blocks != 0).any(axis=(1, 3))


def _build_program(occ_key: bytes, tok: int, variant: str = ""):
    """Build + compile the per-core Bass program.

    occ_key: packed bool (N_OB, N_IB) block occupancy.
    tok: tokens per core (2048 for the real problem).
    variant: ablation flags, comma-separated subset of
      {nomm, noydma, noxdma, f32mm, nowprep}.
    """
    flags = set(variant.split(",")) if variant else set()
    mm_dtype = mybir.dt.float32 if "f32mm" in flags else MM_DTYPE
    occ = np.frombuffer(occ_key, dtype=bool).reshape(N_OB, N_IB)
    n_tt = tok // P  # token tiles

    # Per out-block: contiguous span of occupied if-chunks (band).
    ob_band = {}
    for ob in range(N_OB):
        ibs = np.where(occ[ob])[0]
        if len(ibs):
            ob_band[ob] = (int(ibs.min()), int(ibs.max()))
    # Per of-group: occupied if-chunks (union over the group's 4 out-blocks).
    g_ibs = {}
    for g in range(N_G):
        s = sorted({ib for ob in range(g * OB_PER_G, (g + 1) * OB_PER_G)
                    for ib in np.where(occ[ob])[0].tolist()})
        g_ibs[g] = s
    # Column slot in the WmT store for each (g, ib) pair.
    pair_slot = {}
    for g in range(N_G):
        for ib in g_ibs[g]:
            pair_slot[(g, ib)] = len(pair_slot)
    n_pairs = len(pair_slot)

    nc = bacc.Bacc("TRN2", target_bir_lowering=False, debug=False,
                   num_devices=N_CORES)
    f32 = mybir.dt.float32
    x_d = nc.dram_tensor("x", [tok, IN_F], f32, kind="ExternalInput").ap()
    if "hostwm" in flags:
        w_d = nc.dram_tensor("wm", [OUT_F, IN_F], f32,
                             kind="ExternalInput").ap()
        m_d = None
    else:
        w_d = nc.dram_tensor("w", [OUT_F, IN_F], f32,
                             kind="ExternalInput").ap()
        m_d = nc.dram_tensor("m", [OUT_F, IN_F], f32,
                             kind="ExternalInput").ap()
    b_d = nc.dram_tensor("b", [P, OUT_F], f32, kind="ExternalInput").ap()
    y_d = nc.dram_tensor("y", [tok, OUT_F], f32, kind="ExternalOutput").ap()

    with TileContext(nc) as tc:
        deep = "deep" in flags
        with (
            tc.tile_pool(name="const", bufs=1) as const_pool,
            tc.tile_pool(name="wprep", bufs=3) as wprep_pool,
            tc.tile_pool(name="xio", bufs=4 if deep else 3) as xio_pool,
            tc.tile_pool(name="yio", bufs=4 if deep else 2) as yio_pool,
            tc.tile_pool(name="psum_tr", bufs=2, space="PSUM") as ptr_pool,
            tc.tile_pool(name="psum_mm",
                         bufs=4 if (deep and "ev2" not in flags) else 3,
                         space="PSUM") as pmm_pool,
        ):
            id_dtype = mybir.dt.bfloat16 if "bf16id" in flags else f32
            identity = const_pool.tile([P, P], id_dtype)
            make_identity(nc, identity[:])

            bias_bc = const_pool.tile([P, OUT_F], f32)
            nc.sync.dma_start(out=bias_bc[:], in_=b_d[:, :])

            zblk = const_pool.tile([P, P], f32)
            nc.vector.memset(zblk[:], 0.0)

            # Persistent store of transposed masked weights:
            # column slot k holds WmT[if=128, of=512] for pair k.
            # Stored as MM_DTYPE: the PSUM->SBUF copy rounds to fp32r,
            # which the fp32r matmul requires of its producers.
            wmT = const_pool.tile([P, n_pairs * NG_COLS], mm_dtype)

            # ---- W prep: load occupied bands, mask, transpose ----
            if "gload" in flags and "nowprep" not in flags:
                # one coalesced DMA per of-group: 4 out-blocks stacked
                # along the free dim, covering the group's band union
                for g in range(N_G):
                    ibs = g_ibs[g]
                    if not ibs:
                        continue
                    gib0, gib1 = min(ibs), max(ibs)
                    nbg = gib1 - gib0 + 1
                    wblk = wprep_pool.tile([P, OB_PER_G * nbg * P], f32,
                                           tag="wblk")
                    src_w = w_d[g * NG_COLS:(g + 1) * NG_COLS,
                                gib0 * P:(gib1 + 1) * P].rearrange(
                                    "(a p) c -> p (a c)", p=P)
                    nc.sync.dma_start(out=wblk[:], in_=src_w)
                    if m_d is not None:
                        mblk = wprep_pool.tile([P, OB_PER_G * nbg * P], f32,
                                               tag="mblk")
                        src_m = m_d[g * NG_COLS:(g + 1) * NG_COLS,
                                    gib0 * P:(gib1 + 1) * P].rearrange(
                                        "(a p) c -> p (a c)", p=P)
                        nc.sync.dma_start(out=mblk[:], in_=src_m)
                        nc.vector.tensor_mul(wblk[:], wblk[:], mblk[:])
                    for sub in range(OB_PER_G):
                        ob = g * OB_PER_G + sub
                        for ib in ibs:
                            col = pair_slot[(g, ib)] * NG_COLS + sub * P
                            if not occ[ob, ib]:
                                nc.scalar.copy(out=wmT[:, col:col + P],
                                               in_=zblk[:])
                                continue
                            off = (sub * nbg + (ib - gib0)) * P
                            pst = ptr_pool.tile([P, P], f32, tag="ptr")
                            nc.tensor.transpose(
                                pst[:], wblk[:, off:off + P], identity[:])
                            nc.scalar.copy(out=wmT[:, col:col + P],
                                           in_=pst[:])
            else:
                for ob in ([] if "nowprep" in flags else range(N_OB)):
                    if ob not in ob_band:
                        continue
                    ib0, ib1 = ob_band[ob]
                    nb = ib1 - ib0 + 1
                    g, sub = ob // OB_PER_G, ob % OB_PER_G
                    wblk = wprep_pool.tile([P, nb * P], f32, tag="wblk")
                    r0, r1 = ob * P, (ob + 1) * P
                    c0, c1 = ib0 * P, (ib1 + 1) * P
                    wdma = nc.scalar if "wact" in flags else nc.sync
                    wdma.dma_start(out=wblk[:], in_=w_d[r0:r1, c0:c1])
                    if m_d is not None:
                        mblk = wprep_pool.tile([P, nb * P], f32, tag="mblk")
                        wdma.dma_start(out=mblk[:], in_=m_d[r0:r1, c0:c1])
                        nc.vector.tensor_mul(wblk[:], wblk[:], mblk[:])
                    for ib in range(ib0, ib1 + 1):
                        col = pair_slot[(g, ib)] * NG_COLS + sub * P
                        if not occ[ob, ib]:
                            # this 128-col slot of the pair is all-zero
                            nc.scalar.copy(out=wmT[:, col:col + P],
                                           in_=zblk[:])
                            continue
                        pst = ptr_pool.tile([P, P], f32, tag="ptr")
                        nc.tensor.transpose(
                            pst[:], wblk[:, (ib - ib0) * P:(ib - ib0 + 1) * P],
                            identity[:])
                        nc.scalar.copy(out=wmT[:, col:col + P], in_=pst[:])
            # zero slots for (g, ib) pairs where an out-block has no band row
            # (gload covers every (sub, ib in g_ibs) slot already)
            for g in ([] if ("nowprep" in flags or "gload" in flags)
                      else range(N_G)):
                for ib in g_ibs[g]:
                    for sub in range(OB_PER_G):
                        ob = g * OB_PER_G + sub
                        in_band = ob in ob_band and \
                            ob_band[ob][0] <= ib <= ob_band[ob][1]
                        if not in_band:
                            col = pair_slot[(g, ib)] * NG_COLS + sub * P
                            nc.scalar.copy(out=wmT[:, col:col + P],
                                           in_=zblk[:])

            # ---- main loop over token tiles ----
            for t in range(n_tt):
                xtile = xio_pool.tile([P, IN_F], f32, tag="xtile")
                if "noxdma" not in flags:
                    nc.sync.dma_start(out=xtile[:],
                                      in_=x_d[t * P:(t + 1) * P, :])
                else:
                    nc.vector.memset(xtile[:], 0.5)
                xT = xio_pool.tile([P, IN_F], mm_dtype, tag="xT")
                if "xt512" in flags:
                    for q in range(N_IB // 4):
                        pst4 = ptr_pool.tile([P, 4 * P], f32, tag="ptr4")
                        for k in range(4):
                            ib = q * 4 + k
                            nc.tensor.transpose(
                                pst4[:, k * P:(k + 1) * P],
                                xtile[:, ib * P:(ib + 1) * P], identity[:])
                        nc.scalar.copy(out=xT[:, q * 4 * P:(q + 1) * 4 * P],
                                       in_=pst4[:])
                else:
                    for ib in range(N_IB):
                        pst = ptr_pool.tile([P, P], f32, tag="ptr")
                        nc.tensor.transpose(
                            pst[:], xtile[:, ib * P:(ib + 1) * P], identity[:])
                        nc.scalar.copy(out=xT[:, ib * P:(ib + 1) * P],
                                       in_=pst[:])

                ytile = yio_pool.tile([P, OUT_F], f32, tag="ytile")
                if "ev2" in flags:
                    for gp in range(N_G // 2):
                        pmm2 = pmm_pool.tile([P, 2 * NG_COLS], f32,
                                             tag="pmm2")
                        for h in range(2):
                            g = gp * 2 + h
                            ibs = g_ibs[g]
                            for j, ib in enumerate(ibs):
                                sl = pair_slot[(g, ib)] * NG_COLS
                                nc.tensor.matmul(
                                    pmm2[:, h * NG_COLS:(h + 1) * NG_COLS],
                                    xT[:, ib * P:(ib + 1) * P],
                                    wmT[:, sl:sl + NG_COLS],
                                    start=(j == 0),
                                    stop=(j == len(ibs) - 1))
                        c0, c1 = gp * 2 * NG_COLS, (gp + 1) * 2 * NG_COLS
                        nc.vector.tensor_add(
                            ytile[:, c0:c1], pmm2[:], bias_bc[:, c0:c1])
                    ydma = nc.gpsimd if "ygp" in flags else nc.sync
                    if "noydma" not in flags:
                        ydma.dma_start(out=y_d[t * P:(t + 1) * P, :],
                                       in_=ytile[:])
                    continue
                for g in range(N_G):
                    ibs = g_ibs[g]
                    if not ibs:
                        # of-group with no occupied blocks: output is bias
                        nc.vector.tensor_copy(
                            ytile[:, g * NG_COLS:(g + 1) * NG_COLS],
                            bias_bc[:, g * NG_COLS:(g + 1) * NG_COLS])
                        continue
                    if "nomm" in flags:
                        nc.vector.tensor_copy(
                            ytile[:, g * NG_COLS:(g + 1) * NG_COLS],
                            bias_bc[:, g * NG_COLS:(g + 1) * NG_COLS])
                        continue
                    pmm = pmm_pool.tile([P, NG_COLS], f32, tag="pmm")
                    for j, ib in enumerate(ibs):
                        sl = pair_slot[(g, ib)] * NG_COLS
                        nc.tensor.matmul(
                            pmm[:],
                            xT[:, ib * P:(ib + 1) * P],
                            wmT[:, sl:sl + NG_COLS],
                            start=(j == 0), stop=(j == len(ibs) - 1))
                    nc.vector.tensor_add(
                        ytile[:, g * NG_COLS:(g + 1) * NG_COLS],
                        pmm[:],
                        bias_bc[:, g * NG_COLS:(g + 1) * NG_COLS])
                if "noydma" not in flags:
                    ydma = nc.gpsimd if "ygp" in flags else nc.sync
                    ydma.dma_start(out=y_d[t * P:(t + 1) * P, :],
                                   in_=ytile[:])

    nc.compile()
    return nc


def get_program(sparse_mask: np.ndarray, tok: int = S, variant: str = ""):
    occ = _block_occupancy(sparse_mask)
    key = (occ.tobytes(), tok, variant)
    if key not in _program_cache:
        _program_cache[key] = _build_program(occ.tobytes(), tok, variant)
    return _program_cache[key]


def make_in_maps(x, weight, bias, sparse_mask, variant: str = ""):
    flags = set(variant.split(",")) if variant else set()
    bias_bc = np.ascontiguousarray(
        np.broadcast_to(bias.reshape(1, OUT_F), (P, OUT_F))).astype(np.float32)
    if "hostwm" in flags:
        wm = np.ascontiguousarray(
            np.asarray(weight, np.float32) * np.asarray(sparse_mask,
                                                        np.float32))
        base = {"wm": wm, "b": bias_bc}
    else:
        base = {
            "w": np.ascontiguousarray(weight, dtype=np.float32),
            "m": np.ascontiguousarray(sparse_mask, dtype=np.float32),
            "b": bias_bc,
        }
    return [
        {"x": np.ascontiguousarray(x[c], dtype=np.float32), **base}
        for c in range(N_CORES)
    ]


def kernel(x, weight, bias, sparse_mask):
    x = np.asarray(x)
    weight = np.asarray(weight)
    bias = np.asarray(bias)
    sparse_mask = np.asarray(sparse_mask)
    assert x.shape == (B, S, IN_F), x.shape
    assert weight.shape == (OUT_F, IN_F)
    assert sparse_mask.shape == (OUT_F, IN_F)

    nc = get_program(sparse_mask, variant=DEFAULT_VARIANT)
    in_maps = make_in_maps(x, weight, bias, sparse_mask,
                           variant=DEFAULT_VARIANT)
    res = run_bass_kernel_spmd(nc, in_maps, core_ids=list(range(N_CORES)))
    y = np.stack([res.results[c]["y"] for c in range(N_CORES)], axis=0)
    return y.astype(np.float32)
